# revision 32
# baseline (speedup 1.0000x reference)
"""GCN spatial block on 8 TRN2 NeuronCores (Bass/Tile), data-parallel over B*T.

Per-core algorithm (tokens = B*T/8 = 1944, J=17, C=256), all matmuls bf16.
Tokens are processed in groups of 4, one token per 32-partition strip
(strip starts 0/32/64/96 are the only legal engine-op partition bases).

DMA layouts are fully host-prepared so every transfer is a few large
contiguous descriptors per partition line:
  xtp : padded C-major x   [128, 2(kc), nr, gb*G*PS]  (pad cols pre-zeroed)
  xrp : padded row-major x [128, nr, gb, C]           (pad rows pre-zeroed)
  xc  : compact C-major    [128, 2(kc), rows]         (phase-2 residual)
  mrn : mov_rn broadcast-build tile, rg: [rn, gate] per row (host-computed
        pure input functions: row norms and gate sigmoid)
  out : compact C-major bf16 [128, 2, rows]

Phase 1 is software-pipelined over rounds r (gb=18 groups of G=4 tokens):
the PE computes Gram(r+1) and stage-B(r-1) while the DVE/ACT/Pool engines
assemble round r's per-token adjacency:
  at'' = P*cb2*b3(d*rn) + S*cb3*b3(d):  P = relu(G)*colb(rn)*(1+I) (diag
  doubling folded into the blk2 stationary), cb* column broadcasts built by
  PE matmuls, row sums via reduce(P), d = rsqrt(rs) as exp(-0.5*ln(rs+eps))
  on the scalar engine (Ln/Exp share one activation table set).
  stage A: Z = X^T A'' (PE), stage B: h = W^T Z (PE), h cached in SBUF bf16.
BN stats: sum(h^2) accumulates on the square passes (V for cc0, GpSimd for
cc1); sum(h) is recovered exactly as W^T sum(z) from accumulators riding the
z evacuations, with 4 tiny f32 matmuls at the end.  AllReduce of the raw
sums across the 8 cores (tiny).
Phase 2: fused BN+ReLU from cached h (scalar), + residual (V/G), bf16 out.

BN algebra: out = relu(s_c*h_n + b''_c) + x with s_c = gamma*rsqrt(var+eps),
b''_c = beta - s_c*mean (the Linear bias cancels through BN exactly).
"""

import numpy as np

J = 17
CONNECTIONS = {0: [1, 7], 1: [0, 2], 2: [1, 3], 3: [2], 4: [0, 5], 5: [4, 6], 6: [5],
               7: [0, 8], 8: [7, 9, 11, 14], 9: [8, 10], 10: [9], 11: [8, 12],
               12: [11, 13], 13: [12], 14: [8, 15], 15: [14, 16], 16: [15]}

N_CORES = 8
B, T, C = 64, 243, 256
NTOK_TOTAL = B * T            # 15552
NTOK = NTOK_TOTAL // N_CORES  # 1944 tokens per core
G = 4                         # tokens per group (one per 32-partition strip)
PS = 32                       # partition stride per token strip
GPS = G * PS                  # 128 padded cols per group
RGC = G * J                   # 68 compact rows per group
NG = NTOK // G                # 486 groups per core
GB = 18                       # groups per round
NR = NG // GB                 # 27 rounds
ROWS = NTOK * J               # 33048 compact rows per core
XB = 6                        # groups per stage-A/B batch (N = 408 <= 512)
NB = NG // XB                 # 81 batches
GBP = 6                       # groups per Gram PSUM batch
RNDC = GB * RGC               # 1224 compact cols per round
RNDW = GB * GPS               # 2304 padded cols per round
HC = RNDC // 2                # 612 phase-2 chunk cols

_prog_cache = {}


def _build_adj_np():
    a = np.zeros((J, J), np.float32)
    for i, ns in CONNECTIONS.items():
        for j in ns:
            a[i, j] = 1.0
    eye = np.eye(J, dtype=np.float32)
    adj1_base = a + eye
    paths2 = ((a @ a) > 0).astype(np.float32)
    adj2_pure = ((paths2 - a - eye) > 0).astype(np.float32)
    return adj1_base, adj2_pure


def _host_S(adj1, adj2, w1, w2):
    a1b, a2b = _build_adj_np()
    sig = lambda v: 1.0 / (1.0 + np.exp(-np.asarray(v, np.float64)))
    sp = lambda v: np.log1p(np.exp(np.asarray(v, np.float64)))
    A1 = a1b + sig(adj1)
    A2 = a2b + sig(adj2)
    S = sp(w1)[0] * A1 + sp(w2)[0] * A2
    S = 0.5 * (S + S.T)
    return S.astype(np.float32)


def split_excess_waits(nc, limit=1):
    """This toolchain's walrus rejects instructions with too many sync
    waits ("Too many sync wait commands").  Move excess waits onto
    same-engine NoOps inserted just before the instruction (engine
    streams are in-order, so all-waits-must-pass semantics hold)."""
    import concourse.mybir as mybir
    ctrl = ("InstDrain", "InstNoOp", "InstEventSemaphore")
    k = 0
    for f in nc.m.functions:
        for bb in f.blocks:
            newlist = []
            for inst in bb.instructions:
                si = inst.sync_info
                waits = list(si.on_wait) if si and si.on_wait else []
                lim = 1 if type(inst).__name__ in ctrl else limit
                if len(waits) > lim:
                    for w in waits[lim:]:
                        k += 1
                        nop = mybir.InstNoOp(
                            name=f"waitsplit_{k}", ins=[], outs=[])
                        nop.engine = inst.engine
                        nop.sync_info = mybir.SyncInfo(
                            on_wait=[w], on_update=[])
                        newlist.append(nop)
                    si.on_wait = waits[:lim]
                newlist.append(inst)
            bb.instructions = newlist


def _build_program(n_cores=N_CORES, split_waits=True):
    import concourse.bass as bass
    import concourse.tile as tile
    import concourse.mybir as mybir

    nr, gb = NR, GB

    f32 = mybir.dt.float32
    bf16 = mybir.dt.bfloat16
    AF = mybir.ActivationFunctionType
    ALU = mybir.AluOpType

    nc = bass.Bass()

    xtp = nc.dram_tensor("xtp", [128, 2, nr, RNDW], bf16, kind="ExternalInput")
    xrp = nc.dram_tensor("xrp", [128, nr, gb, C], bf16, kind="ExternalInput")
    xc = nc.dram_tensor("xc", [128, 2, ROWS], bf16, kind="ExternalInput")
    mrn_in = nc.dram_tensor("mrn", [128, nr, gb * J], bf16,
                            kind="ExternalInput")
    rg_in = nc.dram_tensor("rg", [128, nr, 2, gb], f32, kind="ExternalInput")
    w_in = nc.dram_tensor("w4", [128, 2, 2, 128], bf16, kind="ExternalInput")
    w32_in = nc.dram_tensor("w32", [128, 2, 2, 128], f32,
                            kind="ExternalInput")
    s_in = nc.dram_tensor("s_tile", [128, J], f32, kind="ExternalInput")
    ik_in = nc.dram_tensor("ik_tile", [128, J], bf16, kind="ExternalInput")
    srow_in = nc.dram_tensor("srow", [128, 1], f32, kind="ExternalInput")
    b1_in = nc.dram_tensor("blk1", [128, 128], bf16, kind="ExternalInput")
    b2_in = nc.dram_tensor("blk2", [128, 128], bf16, kind="ExternalInput")
    eps_in = nc.dram_tensor("eps_tile", [128, 3], f32, kind="ExternalInput")
    gam_in = nc.dram_tensor("gamma2", [128, 2], f32, kind="ExternalInput")
    bet_in = nc.dram_tensor("beta2", [128, 2], f32, kind="ExternalInput")
    outc = nc.dram_tensor("outc", [128, 2, ROWS], bf16, kind="ExternalOutput")

    with tile.TileContext(nc) as tc:
        with (
            tc.tile_pool(name="const", bufs=1) as constp,
            tc.tile_pool(name="hcache", bufs=1) as hcp,
            tc.tile_pool(name="small", bufs=2) as smallp,
            tc.tile_pool(name="stats", bufs=1) as statsp,
            tc.tile_pool(name="gpsum", bufs=2, space="PSUM") as gpsump,
            tc.tile_pool(name="zhpsum", bufs=2, space="PSUM") as zhpsump,
            tc.tile_pool(name="sppsum", bufs=2, space="PSUM") as sppsump,
            tc.tile_pool(name="dram", bufs=1, space="DRAM") as dramp,
        ):
            # ---- constants ----------------------------------------------
            w_sb = constp.tile([128, 2, 2, 128], bf16)   # [e, ec, cc, c']
            nc.sync.dma_start(w_sb[:, :, :, :], w_in[:, :, :, :])
            w32_sb = constp.tile([128, 2, 2, 128], f32)
            nc.sync.dma_start(w32_sb[:, :, :, :], w32_in[:, :, :, :])
            s_sb = constp.tile([128, J], f32)
            nc.sync.dma_start(s_sb[:, :], s_in[:, :])
            ik_sb = constp.tile([128, J], bf16)
            nc.sync.dma_start(ik_sb[:, :], ik_in[:, :])
            srow_sb = constp.tile([128, 1], f32)
            nc.sync.dma_start(srow_sb[:, :], srow_in[:, :])
            b1_sb = constp.tile([128, 128], bf16)
            nc.sync.dma_start(b1_sb[:, :], b1_in[:, :])
            b2_sb = constp.tile([128, 128], bf16)
            nc.sync.dma_start(b2_sb[:, :], b2_in[:, :])
            eps_sb = constp.tile([128, 3], f32)
            nc.sync.dma_start(eps_sb[:, :], eps_in[:, :])
            gam_sb = constp.tile([128, 2], f32)
            nc.sync.dma_start(gam_sb[:, :], gam_in[:, :])
            bet_sb = constp.tile([128, 2], f32)
            nc.sync.dma_start(bet_sb[:, :], bet_in[:, :])

            h_sb = hcp.tile([128, 2, ROWS], bf16)
            st2_sb = statsp.tile([128, 2, NB], f32)   # sum(h^2) per batch
            zs_sb = statsp.tile([128, 2, NB], f32)    # sum(z) per batch

            p1_pools = (
                tc.tile_pool(name="persist", bufs=1),
                tc.tile_pool(name="xin", bufs=2),
                tc.tile_pool(name="asm", bufs=2),
                tc.tile_pool(name="sqj", bufs=1),
                tc.tile_pool(name="zsb", bufs=2),
            )
            perp, xinp, asmp, sqjp, zsbp = [p.__enter__() for p in p1_pools]

            # persistent tiles whose pad regions must stay zero (full-width
            # chain ops read the pad strip rows)
            exp_tl = [perp.tile([128, gb, RGC], bf16, tag=f"expp{p}",
                                name=f"exp_tl{p}") for p in range(2)]
            gc_tl = [perp.tile([128, gb * J], bf16, tag=f"gcp{p}",
                               name=f"gc_tl{p}") for p in range(2)]
            for p in range(2):
                nc.gpsimd.memset(exp_tl[p][:, :, :], 0.0)
                nc.gpsimd.memset(gc_tl[p][:, :], 0.0)

            def b3(ap2d):
                """[128, gb] AP -> [128, gb, J] broadcast (step-0 on J)."""
                return ap2d.rearrange("p gg -> p gg ()").broadcast_to(
                    (128, gb, J))

            def k3(tl2d):
                """[128, J] const tile -> [128, gb, J] broadcast (step-0 g)."""
                return tl2d[:, :].rearrange("p b -> p () b").broadcast_to(
                    (128, gb, J))

            def cview(tl):
                return tl[:, :].rearrange("p (gg b) -> p gg b", b=J)

            def rsqrt_act(x_ap, name, eps_col, shape=(128, gb)):
                """y = (x+eps)^-1/2 = exp(-0.5*ln(x+eps)) on the scalar
                engine; Ln/Exp live in one activation table set, so no
                per-round table reloads (unlike Sqrt vs Sigmoid)."""
                t_t = smallp.tile(list(shape), f32, tag=f"lt_{name}")
                nc.scalar.activation(t_t[:, :], x_ap, AF.Ln,
                                     bias=eps_sb[:, eps_col:eps_col + 1])
                y_t = smallp.tile(list(shape), f32, tag=f"ly_{name}")
                nc.scalar.activation(y_t[:, :], t_t[:, :], AF.Exp, scale=-0.5)
                return y_t

            live = {}

            def do_loads(r):
                xt_t = xinp.tile([128, 2, gb, GPS], bf16, tag="xt",
                                 name="xt_t")
                nc.sync.dma_start(
                    xt_t[:, :, :, :],
                    xtp[:, :, r, :].rearrange("p k (g w) -> p k g w", w=GPS))
                xr_t = xinp.tile([128, gb, C], bf16, tag="xr", name="xr_t")
                nc.sync.dma_start(xr_t[:, :, :], xrp[:, r, :, :])
                mrn_t = xinp.tile([128, gb * J], bf16, tag="mrn",
                                  name="mrn_t")
                nc.sync.dma_start(mrn_t[:, :], mrn_in[:, r, :])
                rg_t = xinp.tile([128, 2, gb], f32, tag="rg", name="rg_t")
                nc.sync.dma_start(rg_t[:, :, :], rg_in[:, r, :, :])
                live[r] = {"xt": xt_t, "xr": xr_t, "mrn": mrn_t, "rg": rg_t}

            def do_gram(r):
                xt_t = live[r]["xt"]
                gc3 = cview(gc_tl[r % 2])
                for hf in range(gb // GBP):
                    g_ps = gpsump.tile([128, GBP, 128], f32, tag="gram",
                                       name="g_ps")
                    for gi in range(GBP):
                        g = hf * GBP + gi
                        for kc in range(2):
                            stat = xt_t[:, kc, g, :].opt()
                            nc.tensor.matmul(
                                g_ps[:, gi, :], stat, stat,
                                start=(kc == 0), stop=(kc == 1))
                    for t in range(G):
                        src = g_ps[PS * t:PS * t + J, :, PS * t:PS * t + J]
                        dst = gc3[PS * t:PS * t + J,
                                  hf * GBP:(hf + 1) * GBP, :]
                        if (hf * G + t) % 3 == 0:
                            nc.vector.tensor_scalar_max(dst, src, 0.0)
                        else:
                            nc.scalar.activation(dst, src, AF.Relu)

            def do_chain(r):
                L = live[r]
                rn_v = L["rg"][:, 0, :]
                gsig_v = L["rg"][:, 1, :]
                gc3 = cview(gc_tl[r % 2])

                # column broadcast of rn with diagonal doubling (blk2)
                cbrn_ps = sppsump.tile([128, gb * J], f32, tag="sp",
                                       name="cbrn_ps")
                nc.tensor.matmul(cbrn_ps[:, :], b2_sb[:, :], L["mrn"][:, :],
                                 start=True, stop=True)
                # P = relu(G) * colb(rn) * (1+I); row sums fold the dyn +1
                pp_t = asmp.tile([128, gb * J], bf16, tag="pp", name="pp_t")
                nc.vector.tensor_tensor(cview(pp_t), gc3, cview(cbrn_ps),
                                        ALU.mult)
                wsum_t = smallp.tile([128, gb], f32, tag="wsum")
                nc.vector.tensor_reduce(
                    wsum_t[:, :], cview(pp_t), mybir.AxisListType.X, ALU.add)

                # rowsums rs = gsig*Srow + (1-gsig)*rn*wsum, d = rsqrt(rs)
                a_t = smallp.tile([128, gb], f32, tag="ra")
                nc.vector.tensor_tensor(a_t[:, :], rn_v, wsum_t[:, :],
                                        ALU.mult)
                bb_t = smallp.tile([128, gb], f32, tag="rb")
                nc.vector.tensor_tensor(
                    bb_t[:, :], srow_sb[:, 0:1].broadcast_to((128, gb)),
                    a_t[:, :], ALU.subtract)
                rs_t = smallp.tile([128, gb], f32, tag="rs")
                nc.vector.tensor_tensor(rs_t[:, :], gsig_v, bb_t[:, :],
                                        ALU.mult)
                nc.vector.tensor_tensor(rs_t[:, :], rs_t[:, :], a_t[:, :],
                                        ALU.add)
                d_t = rsqrt_act(rs_t[:, :], "d", 1)

                u1_t = smallp.tile([128, gb], f32, tag="u1")
                nc.vector.tensor_tensor(u1_t[:, :], d_t[:, :], rn_v, ALU.mult)
                m3_t = smallp.tile([128, gb], f32, tag="m3")
                nc.vector.tensor_tensor(m3_t[:, :], d_t[:, :], gsig_v,
                                        ALU.mult)
                m2_t = smallp.tile([128, gb], f32, tag="m2")
                nc.vector.tensor_tensor(m2_t[:, :], d_t[:, :], m3_t[:, :],
                                        ALU.subtract)

                mov2_t = asmp.tile([128, gb * J], bf16, tag="mov2",
                                   name="mov2_t")
                nc.gpsimd.tensor_tensor(cview(mov2_t), k3(ik_sb),
                                        b3(m2_t[:, :]), ALU.mult)
                cb2_ps = sppsump.tile([128, gb * J], f32, tag="sp",
                                      name="cb2_ps")
                nc.tensor.matmul(cb2_ps[:, :], b1_sb[:, :], mov2_t[:, :],
                                 start=True, stop=True)
                mov3_t = asmp.tile([128, gb * J], bf16, tag="mov3",
                                   name="mov3_t")
                nc.gpsimd.tensor_tensor(cview(mov3_t), k3(ik_sb),
                                        b3(m3_t[:, :]), ALU.mult)
                cb3_ps = sppsump.tile([128, gb * J], f32, tag="sp",
                                      name="cb3_ps")
                nc.tensor.matmul(cb3_ps[:, :], b1_sb[:, :], mov3_t[:, :],
                                 start=True, stop=True)

                e1_t = asmp.tile([128, gb * J], bf16, tag="e1", name="e1_t")
                nc.vector.tensor_tensor(cview(e1_t), cview(pp_t),
                                        cview(cb2_ps), ALU.mult)
                nc.gpsimd.tensor_tensor(cview(e1_t), cview(e1_t),
                                        b3(u1_t[:, :]), ALU.mult)
                e2_t = asmp.tile([128, gb * J], bf16, tag="e2", name="e2_t")
                nc.vector.tensor_tensor(cview(e2_t), k3(s_sb), cview(cb3_ps),
                                        ALU.mult)
                nc.gpsimd.tensor_tensor(cview(e2_t), cview(e2_t),
                                        b3(d_t[:, :]), ALU.mult)
                at_t = asmp.tile([128, gb * J], bf16, tag="at", name="at_t")
                nc.gpsimd.tensor_tensor(cview(at_t), cview(e1_t),
                                        cview(e2_t), ALU.add)

                exp_t = exp_tl[r % 2]
                for t in range(G):
                    dst = exp_t[PS * t:PS * t + J, :, J * t:J * (t + 1)]
                    src = cview(at_t)[PS * t:PS * t + J, :, :]
                    if t % 2 == 0:
                        nc.vector.tensor_copy(dst, src)
                    else:
                        nc.gpsimd.tensor_copy(dst, src)

            def do_stage_a(r):
                xr_t = live[r]["xr"]
                exp_t = exp_tl[r % 2]
                z_sb = zsbp.tile([128, 2, RNDC], bf16, tag="zsb",
                                 name="z_sb")
                live[r]["z"] = z_sb
                for bi in range(gb // XB):
                    zc = slice(bi * XB * RGC, (bi + 1) * XB * RGC)
                    bidx = r * (gb // XB) + bi
                    for ec in range(2):
                        z_ps = zhpsump.tile([128, 512], f32, tag="zh",
                                            name=f"zps{ec}")
                        for xi in range(XB):
                            g = bi * XB + xi
                            nc.tensor.matmul(
                                z_ps[:, xi * RGC:(xi + 1) * RGC],
                                xr_t[:, g, ec * 128:(ec + 1) * 128],
                                exp_t[:, g, :],
                                start=True, stop=True)
                        if ec == 0:
                            nc.scalar.activation(
                                z_sb[:, ec, zc].opt(), z_ps[:, 0:XB * RGC],
                                AF.Copy,
                                accum_out=zs_sb[:, ec, bidx:bidx + 1].opt())
                        else:
                            nc.vector.tensor_scalar(
                                z_sb[:, ec, zc].opt(), z_ps[:, 0:XB * RGC],
                                1.0, 0.0, ALU.mult, ALU.add,
                                accum_out=zs_sb[:, ec, bidx:bidx + 1].opt())

            def do_stage_b(r):
                z_sb = live[r].pop("z")
                for bi in range(gb // XB):
                    zc = slice(bi * XB * RGC, (bi + 1) * XB * RGC)
                    cols = slice(r * RNDC + bi * XB * RGC,
                                 r * RNDC + (bi + 1) * XB * RGC)
                    bidx = r * (gb // XB) + bi
                    for cc in range(2):
                        h_ps = zhpsump.tile([128, 512], f32, tag="zh",
                                            name=f"hps{cc}")
                        for ec in range(2):
                            nc.tensor.matmul(
                                h_ps[:, 0:XB * RGC],
                                w_sb[:, ec, cc, :],
                                z_sb[:, ec, zc],
                                start=(ec == 0), stop=(ec == 1))
                        hv = h_sb[:, cc, cols].opt()
                        if cc == 0:
                            nc.scalar.copy(hv, h_ps[:, 0:XB * RGC])
                            sq_t = sqjp.tile([128, XB * RGC], bf16,
                                             tag="sqv", name="sqv_t")
                            nc.vector.scalar_tensor_tensor(
                                sq_t[:, :], hv, 1.0, hv,
                                ALU.bypass, ALU.mult,
                                accum_out=st2_sb[:, cc, bidx:bidx + 1].opt())
                        else:
                            nc.vector.tensor_copy(hv, h_ps[:, 0:XB * RGC])
                            nc.scalar.activation(
                                h_ps[:, 0:XB * RGC], h_ps[:, 0:XB * RGC],
                                AF.Square,
                                accum_out=st2_sb[:, cc, bidx:bidx + 1].opt())
                del live[r]

            # ================= PHASE 1 (software-pipelined) =============
            do_loads(0)
            do_gram(0)
            for r in range(nr):
                if r + 1 < nr:
                    do_loads(r + 1)
                    do_gram(r + 1)
                if r >= 1:
                    do_stage_b(r - 1)
                do_chain(r)
                do_stage_a(r)
            do_stage_b(nr - 1)
            for p in reversed(p1_pools):
                p.__exit__(None, None, None)

            # ================= BN STATS + ALLREDUCE =====================
            # sum(h) = W^T sum(z) (4 tiny f32 matmuls), sum(h^2) from st2
            zsum_t = smallp.tile([128, 2], f32, tag="zsum")
            for ec in range(2):
                nc.vector.tensor_reduce(
                    zsum_t[:, ec:ec + 1], zs_sb[:, ec, :].opt(),
                    mybir.AxisListType.X, ALU.add)
            ar_t = smallp.tile([128, 4], f32, tag="ar")
            ar3 = ar_t[:, :].rearrange("p (k two) -> p k two", two=2)
            msum_ps = zhpsump.tile([128, 2], f32, tag="zh", name="msum_ps")
            for cc in range(2):
                for ec in range(2):
                    nc.tensor.matmul(
                        msum_ps[:, cc:cc + 1], w32_sb[:, ec, cc, :],
                        zsum_t[:, ec:ec + 1],
                        start=(ec == 0), stop=(ec == 1))
                nc.vector.tensor_reduce(
                    ar3[:, cc, 1:2], st2_sb[:, cc, :].opt(),
                    mybir.AxisListType.X, ALU.add)
            nc.vector.tensor_copy(ar3[:, 0, 0:1], msum_ps[:, 0:1])
            nc.vector.tensor_copy(ar3[:, 1, 0:1], msum_ps[:, 1:2])
            arin_d = dramp.tile([128, 4], f32)
            arout_d = dramp.tile([128, 4], f32)
            nc.sync.dma_start(arin_d[:, :], ar_t[:, :])
            nc.gpsimd.collective_compute(
                "AllReduce", ALU.add,
                replica_groups=[list(range(n_cores))],
                ins=[arin_d.opt()], outs=[arout_d.opt()])
            arg_t = smallp.tile([128, 4], f32, tag="arg")
            nc.sync.dma_start(arg_t[:, :], arout_d[:, :])
            arg3 = arg_t[:, :].rearrange("p (k two) -> p k two", two=2)

            sc_t = constp.tile([128, 2], f32)
            bpp_t = constp.tile([128, 2], f32)
            vtmp = smallp.tile([128, 2], f32, tag="vtmp")
            nc.vector.tensor_scalar_mul(arg_t[:, :], arg_t[:, :],
                                        1.0 / (n_cores * ROWS))
            for cc in range(2):
                nc.vector.tensor_tensor(vtmp[:, cc:cc + 1], arg3[:, cc, 0:1],
                                        arg3[:, cc, 0:1], ALU.mult)
                nc.vector.tensor_tensor(vtmp[:, cc:cc + 1], arg3[:, cc, 1:2],
                                        vtmp[:, cc:cc + 1], ALU.subtract)
            rst_t = rsqrt_act(vtmp[:, :], "bn", 2, shape=(128, 2))
            nc.vector.tensor_tensor(sc_t[:, :], rst_t[:, :], gam_sb[:, :],
                                    ALU.mult)
            for cc in range(2):
                nc.vector.tensor_tensor(bpp_t[:, cc:cc + 1], sc_t[:, cc:cc + 1],
                                        arg3[:, cc, 0:1], ALU.mult)
            nc.vector.tensor_tensor(bpp_t[:, :], bet_sb[:, :], bpp_t[:, :],
                                    ALU.subtract)

            # ================= PHASE 2 ==================================
            def do_phase2(p2p):
                for i in range(ROWS // HC):
                    cols = slice(i * HC, (i + 1) * HC)
                    res_t = p2p.tile([128, 2, HC], bf16, tag="res",
                                     name="res_t")
                    nc.sync.dma_start(res_t[:, :, :], xc[:, :, cols])
                    out_t = p2p.tile([128, 2, HC], bf16, tag="out",
                                     name="out_t")
                    for cc in range(2):
                        nc.scalar.activation(
                            out_t[:, cc, :], h_sb[:, cc, cols].opt(),
                            AF.Relu, bias=bpp_t[:, cc:cc + 1],
                            scale=sc_t[:, cc:cc + 1])
                        if cc == 0:
                            nc.vector.tensor_tensor(out_t[:, cc, :],
                                                    out_t[:, cc, :],
                                                    res_t[:, cc, :], ALU.add)
                        else:
                            nc.gpsimd.tensor_tensor(out_t[:, cc, :],
                                                    out_t[:, cc, :],
                                                    res_t[:, cc, :], ALU.add)
                    nc.sync.dma_start(outc[:, :, cols], out_t[:, :, :])

            with tc.tile_pool(name="p2", bufs=6) as p2p:
                do_phase2(p2p)



    if split_waits:
        split_excess_waits(nc)
    return nc


def _get_program():
    if "nc" not in _prog_cache:
        _prog_cache["nc"] = _build_program()
    return _prog_cache["nc"]


def make_core_inputs(x_shard, W, gate_w, gate_b, S, bn_gamma, bn_beta):
    """Build the per-core in_map. x_shard: [NTOK, J, C] f32."""
    import ml_dtypes
    bf = ml_dtypes.bfloat16
    xb = x_shard.astype(bf)                               # [NTOK, J, C]

    # padded C-major: [128, 2, nr, gb*G*PS], pad joint cols zeroed
    x5 = xb.reshape(NR, GB, G, J, C)
    xpad = np.zeros((NR, GB, G, PS, C), bf)
    xpad[:, :, :, :J, :] = x5
    xtp = np.ascontiguousarray(
        xpad.transpose(4, 0, 1, 2, 3).reshape(2, 128, NR, RNDW)
        .transpose(1, 0, 2, 3))

    # padded row-major: [128, nr, gb, C]  (partition = 32*t + b, pads zero)
    xrp = np.zeros((4, PS, NR, GB, C), bf)
    xrp[:, :J] = x5.transpose(2, 3, 0, 1, 4)
    xrp = np.ascontiguousarray(xrp.reshape(128, NR, GB, C))

    # compact C-major: [128, 2, rows]
    xcm = np.ascontiguousarray(
        xb.reshape(ROWS, 2, 128).transpose(2, 1, 0))

    # host-computed pure input functions: row norms and gate sigmoid
    xf32 = x_shard.astype(np.float32)
    rn = 1.0 / np.maximum(np.linalg.norm(xf32, axis=-1), 1e-12)  # [NTOK, J]
    glog = xf32.reshape(-1, C) @ gate_w.astype(np.float32).reshape(C, 1)
    gate = 1.0 / (1.0 + np.exp(-(glog.reshape(NTOK, J) + gate_b)))
    rn5 = rn.reshape(NR, GB, G, J)
    gt5 = gate.reshape(NR, GB, G, J)
    rgp = np.zeros((4, PS, NR, 2, GB), np.float32)
    rgp[:, :J, :, 0, :] = rn5.transpose(2, 3, 0, 1)
    rgp[:, :J, :, 1, :] = gt5.transpose(2, 3, 0, 1)
    rgp = np.ascontiguousarray(rgp.reshape(128, NR, 2, GB))

    # mov_rn: mrn[32t+b, r, g*J+b] = rn[(r,g,t), b], zero elsewhere
    mrn = np.zeros((4, PS, NR, GB, J), np.float32)
    for j in range(J):
        mrn[:, j, :, :, j] = rn5[:, :, :, j].transpose(2, 0, 1)
    mrn = np.ascontiguousarray(mrn.reshape(128, NR, GB * J).astype(bf))

    w4f = W.astype(np.float32).reshape(2, 128, 2, 128).transpose(1, 0, 2, 3)
    w4 = np.ascontiguousarray(w4f.astype(bf))
    # f32 copy must match the bf16 weights the real matmuls used
    w32 = np.ascontiguousarray(w4.astype(np.float32))

    s_tile = np.zeros((128, J), np.float32)
    i_tile = np.zeros((128, J), np.float32)
    srow = np.ones((128, 1), np.float32)
    blk1 = np.zeros((128, 128), np.float32)
    for t in range(G):
        s_tile[PS * t:PS * t + J, :] = S
        i_tile[PS * t:PS * t + J, :] = np.eye(J, dtype=np.float32)
        srow[PS * t:PS * t + J, 0] = S.sum(1)
        blk1[PS * t:PS * t + J, PS * t:PS * t + J] = 1.0
    blk2 = blk1 + np.eye(128, dtype=np.float32)
    return {
        "xtp": xtp,
        "xrp": xrp,
        "xc": xcm,
        "mrn": mrn,
        "rg": rgp,
        "w4": w4,
        "w32": w32,
        "s_tile": s_tile,
        "ik_tile": i_tile.astype(bf),
        "srow": srow,
        "blk1": blk1.astype(bf),
        "blk2": blk2.astype(bf),
        "eps_tile": np.tile(np.array([1e-20, 1e-6, 1e-5], np.float32),
                            (128, 1)),
        "gamma2": np.ascontiguousarray(bn_gamma.reshape(2, 128).T),
        "beta2": np.ascontiguousarray(bn_beta.reshape(2, 128).T),
    }


def kernel(**inputs):
    x = np.asarray(inputs["x"], np.float32)
    W = np.asarray(inputs["W"], np.float32)
    gate_w = np.asarray(inputs["gate_w"], np.float32)
    gate_b = float(np.asarray(inputs["gate_b"]).reshape(-1)[0])
    bn_gamma = np.asarray(inputs["bn_gamma"], np.float32)
    bn_beta = np.asarray(inputs["bn_beta"], np.float32)
    S = _host_S(np.asarray(inputs["adj_learnable_1st"], np.float32),
                np.asarray(inputs["adj_learnable_2nd"], np.float32),
                np.asarray(inputs["weight_static_1st"], np.float32),
                np.asarray(inputs["weight_static_2nd"], np.float32))

    xf = x.reshape(NTOK_TOTAL, J, C)
    in_maps = []
    for c in range(N_CORES):
        shard = xf[c * NTOK:(c + 1) * NTOK]
        in_maps.append(make_core_inputs(shard, W, gate_w, gate_b, S,
                                        bn_gamma, bn_beta))

    from concourse.bass_utils import run_bass_kernel_spmd
    nc = _get_program()
    res = run_bass_kernel_spmd(nc, in_maps, core_ids=list(range(N_CORES)))
    _prog_cache["last_result"] = res

    out = np.empty((NTOK_TOTAL, J, C), np.float32)
    for c in range(N_CORES):
        oc = np.asarray(res.results[c]["outc"], dtype=np.float32)
        out[c * NTOK:(c + 1) * NTOK] = (
            oc.transpose(2, 1, 0).reshape(NTOK, J, C))
    return out.reshape(B, T, J, C)
